# revision 3
# baseline (speedup 1.0000x reference)
"""GRU kernel for Trainium2 (8 NeuronCores, data-parallel over batch).

Problem shapes (hardcoded): x [B=256, T=512, D=256], w_i [256, 1536],
w_h [512, 1536], b_i/b_h [1536]; out [256, 512, 512] fp32.

Strategy:
  - Shard batch B across 8 cores (32 samples each), replicate weights.
  - On-chip layout is "transposed/folded": the 3H gate axis lives on the
    128 partitions (12 m-tiles of 128), batch on the free axis, so that
    the per-step gate arithmetic uses all 128 vector lanes.
  - Per recurrence step: hg = h @ w_h via 48 bf16 matmuls with w_h tiles
    as the (self-loading, FWL) stationary operand and h^T [128, 32] as
    the moving operand, accumulating into PSUM.
  - xg = x @ w_i (+ biases) for the NEXT 32-step chunk is computed by
    big-N matmuls interleaved into the PE stream so it hides in the
    dependency-wait gaps of the sequential recurrence.
  - Host pre-transposes x / folds weights to bf16 and un-transposes the
    output (not counted in HW time; cheap numpy).
"""

import sys

if "/opt/trn_rl_repo" not in sys.path:
    sys.path.insert(0, "/opt/trn_rl_repo")

from contextlib import ExitStack

import ml_dtypes
import numpy as np

import concourse.bass as bass
import concourse.tile as tile
from concourse import bacc, mybir
from concourse.bass_utils import run_bass_kernel_spmd

B, T, D, H = 256, 512, 256, 512
NCORES = 8
BL = B // NCORES  # 32 samples per core
G3 = 3 * H  # 1536
MT = G3 // 128  # 12 m-tiles over the gate axis
KB = H // 128  # 4 fold blocks of h (contraction for w_h)
KD = D // 128  # 2 fold blocks of d (contraction for w_i)

F32 = mybir.dt.float32
BF16 = mybir.dt.bfloat16
AF = mybir.ActivationFunctionType
BF16NP = ml_dtypes.bfloat16

_cache = {}


def _build_program(t_steps: int, tc: int, with_bhn: bool):
    """Emit + compile the SPMD Bass program for one core's shard."""
    assert t_steps % tc == 0
    nchunk = t_steps // tc
    # xg GEMM over one chunk: moving N per matmul
    ncols_chunk = tc * BL
    XN = min(512, ncols_chunk)
    ncol = ncols_chunk // XN  # N-slices per (chunk, m-tile)
    tper = XN // BL  # timesteps covered by one N-slice

    nc = bacc.Bacc(
        "TRN2",
        target_bir_lowering=False,
        debug=False,
        enable_asserts=False,
        num_devices=NCORES,
    )

    xt_d = nc.dram_tensor("xt", [KD, 128, t_steps * BL], BF16, kind="ExternalInput").ap()
    whp_d = nc.dram_tensor("whp", [KB, 128, G3], BF16, kind="ExternalInput").ap()
    wip_d = nc.dram_tensor("wip", [KD, 128, G3], BF16, kind="ExternalInput").ap()
    xb_d = nc.dram_tensor("xb", [128, MT], F32, kind="ExternalInput").ap()
    bhn_d = nc.dram_tensor("bhn", [128, KB], F32, kind="ExternalInput").ap()
    out_d = nc.dram_tensor("outT", [KB, 128, t_steps, BL], F32, kind="ExternalOutput").ap()

    with tile.TileContext(nc) as tc_ctx, ExitStack() as ctx:
        consts = ctx.enter_context(tc_ctx.tile_pool(name="consts", bufs=1))
        xtp = ctx.enter_context(tc_ctx.tile_pool(name="xt", bufs=3))
        xgp = ctx.enter_context(tc_ctx.tile_pool(name="xg", bufs=2))
        outp = ctx.enter_context(tc_ctx.tile_pool(name="outc", bufs=2))
        hbfp = ctx.enter_context(tc_ctx.tile_pool(name="hbf", bufs=3))
        tmps = ctx.enter_context(tc_ctx.tile_pool(name="tmps", bufs=3))
        ps_zr = ctx.enter_context(tc_ctx.tile_pool(name="ps_zr", bufs=2, space="PSUM"))
        ps_n = ctx.enter_context(tc_ctx.tile_pool(name="ps_n", bufs=2, space="PSUM"))
        ps_xg = ctx.enter_context(tc_ctx.tile_pool(name="ps_xg", bufs=2, space="PSUM"))

        # --- resident constants -------------------------------------------
        wh_sb = consts.tile([128, KB, G3], BF16)
        wi_sb = consts.tile([128, KD, G3], BF16)
        xb_sb = consts.tile([128, MT], F32)
        bhn_sb = consts.tile([128, KB], F32)
        zero_f32 = consts.tile([128, KB, BL], F32)
        zero_bf = consts.tile([128, KB, BL], BF16)

        for kb in range(KB):
            nc.sync.dma_start(wh_sb[:, kb, :], whp_d[kb])
        for kd in range(KD):
            nc.sync.dma_start(wi_sb[:, kd, :], wip_d[kd])
        nc.sync.dma_start(xb_sb[:], xb_d[:])
        nc.sync.dma_start(bhn_sb[:], bhn_d[:])
        nc.vector.memset(zero_f32[:], 0.0)
        nc.vector.memset(zero_bf[:], 0.0)

        # ---- xg GEMM emitter (one (mt, j) N-slice of a chunk) ------------
        def emit_xg_group(xg_tile, xt_tile, mt, j):
            pt = ps_xg.tile([128, XN], F32, tag="psxg")
            for kd in range(KD):
                nc.tensor.matmul(
                    pt[:],
                    lhsT=wi_sb[:, kd, mt * 128 : (mt + 1) * 128],
                    rhs=xt_tile[:, kd, j * XN : (j + 1) * XN],
                    start=(kd == 0),
                    stop=(kd == KD - 1),
                )
            # evacuate + bias (b_i, plus b_h for the z/r tiles) -> bf16
            nc.scalar.activation(
                xg_tile[:, mt, j * tper : (j + 1) * tper, :],
                pt[:],
                AF.Identity,
                bias=xb_sb[:, mt : mt + 1],
            )

        def load_xt(c):
            t0 = c * tc
            xt_tile = xtp.tile([128, KD, ncols_chunk], BF16, tag="xt")
            for kd in range(KD):
                nc.sync.dma_start(
                    xt_tile[:, kd, :], xt_d[kd, :, t0 * BL : (t0 + tc) * BL]
                )
            return xt_tile

        # preamble: xt for chunks 0/1, xg for chunk 0
        xt_cur = load_xt(0)
        xt_next = load_xt(1) if nchunk > 1 else None
        xg_cur = xgp.tile([128, MT, tc, BL], BF16, tag="xg")
        for mt in range(MT):
            for j in range(ncol):
                emit_xg_group(xg_cur, xt_cur, mt, j)

        h_bf = zero_bf
        h_f32 = zero_f32
        outc_prev = None

        for c in range(nchunk):
            outc = outp.tile([128, KB, tc, BL], F32, tag="outc")
            # schedule of next-chunk xg groups to interleave into this chunk
            groups = [(mt, j) for mt in range(MT) for j in range(ncol)]
            if c + 1 < nchunk:
                xg_next = xgp.tile([128, MT, tc, BL], BF16, tag="xg")
                xt_after = load_xt(c + 2) if c + 2 < nchunk else None
            else:
                xg_next = None

            for tt in range(tc):
                # ---- hg = h @ w_h into PSUM (z,r tiles first, then n) ----
                pzr = ps_zr.tile([128, 8, BL], F32, tag="pzr")
                pn = ps_n.tile([128, KB, BL], F32, tag="pn")
                for mt in range(MT):
                    dst = pzr[:, mt, :] if mt < 8 else pn[:, mt - 8, :]
                    for kb in range(KB):
                        nc.tensor.matmul(
                            dst,
                            lhsT=wh_sb[:, kb, mt * 128 : (mt + 1) * 128],
                            rhs=h_bf[:, kb, :],
                            start=(kb == 0),
                            stop=(kb == KB - 1),
                        )
                # interleave next-chunk xg work into the PE stream
                if xg_next is not None:
                    lo = len(groups) * tt // tc
                    hi = len(groups) * (tt + 1) // tc
                    for mt, j in groups[lo:hi]:
                        emit_xg_group(xg_next, xt_next, mt, j)

                # ---- gates -----------------------------------------------
                zrpre = tmps.tile([128, 8, BL], F32, tag="zrpre")
                zr = tmps.tile([128, 8, BL], F32, tag="zr")
                zc = tmps.tile([128, KB, BL], F32, tag="zc")
                tn = tmps.tile([128, KB, BL], F32, tag="tn")
                npre = tmps.tile([128, KB, BL], F32, tag="npre")
                nv = tmps.tile([128, KB, BL], F32, tag="nv")
                zh = tmps.tile([128, KB, BL], F32, tag="zh")
                w1 = tmps.tile([128, KB, BL], F32, tag="w1")

                nc.vector.tensor_add(zrpre[:], pzr[:], xg_cur[:, 0:8, tt, :])
                nc.scalar.activation(zr[:], zrpre[:], AF.Sigmoid)
                # 1 - z == sigmoid(-pre)
                nc.scalar.activation(zc[:], zrpre[:, 0:4, :], AF.Sigmoid, scale=-1.0)
                if with_bhn:
                    for kb in range(KB):
                        nc.vector.scalar_tensor_tensor(
                            tn[:, kb, :],
                            pn[:, kb, :],
                            bhn_sb[:, kb : kb + 1],
                            zr[:, 4 + kb, :],
                            op0=mybir.AluOpType.add,
                            op1=mybir.AluOpType.mult,
                        )
                else:
                    nc.vector.tensor_mul(tn[:], pn[:], zr[:, 4:8, :])
                nc.vector.tensor_add(npre[:], tn[:], xg_cur[:, 8:12, tt, :])
                nc.vector.tensor_mul(zh[:], zr[:, 0:4, :], h_f32[:])
                nc.scalar.activation(nv[:], npre[:], AF.Tanh)
                nc.vector.tensor_mul(w1[:], zc[:], nv[:])
                h_f32 = outc[:, :, tt, :]
                nc.vector.tensor_add(h_f32, w1[:], zh[:])
                h_bf = hbfp.tile([128, KB, BL], BF16, tag="hbf")
                nc.scalar.activation(h_bf[:], h_f32, AF.Copy)

            # ---- store chunk, rotate buffers ----
            t0 = c * tc
            for kb in range(KB):
                nc.sync.dma_start(out_d[kb, :, t0 : t0 + tc, :], outc[:, kb, :, :])
            outc_prev = outc  # noqa: F841  (keeps tile alive conceptually)
            xg_cur = xg_next
            xt_cur = xt_next
            if c + 1 < nchunk:
                xt_next = xt_after

    nc.compile()
    return nc


def _get_program(t_steps: int, tc: int, with_bhn: bool):
    key = (t_steps, tc, with_bhn)
    if key not in _cache:
        _cache[key] = _build_program(t_steps, tc, with_bhn)
    return _cache[key]


def _prep_core_inputs(x, w_i, b_i, w_h, b_h, core, t_steps):
    xc = x[core * BL : (core + 1) * BL, :t_steps, :]  # [BL, T, D]
    xt = np.ascontiguousarray(xc.transpose(2, 1, 0)).reshape(KD, 128, t_steps * BL)
    xb = b_i.astype(np.float64).copy()
    xb[: 2 * H] += b_h[: 2 * H].astype(np.float64)
    return {
        "xt": xt.astype(BF16NP),
        "whp": w_h.reshape(KB, 128, G3).astype(BF16NP),
        "wip": w_i.reshape(KD, 128, G3).astype(BF16NP),
        "xb": np.ascontiguousarray(
            xb.astype(np.float32).reshape(MT, 128).T
        ),
        "bhn": np.ascontiguousarray(b_h[2 * H :].reshape(KB, 128).T.astype(np.float32)),
    }


def run(x, w_i, b_i, w_h, b_h, t_steps=T, tc=32, trace=False):
    with_bhn = bool(np.any(b_h[2 * H :] != 0))
    nc = _get_program(t_steps, tc, with_bhn)
    in_maps = [
        _prep_core_inputs(x, w_i, b_i, w_h, b_h, core, t_steps)
        for core in range(NCORES)
    ]
    res = run_bass_kernel_spmd(
        nc, in_maps, core_ids=list(range(NCORES)), trace=trace
    )
    outs = []
    for core in range(NCORES):
        ot = np.asarray(res.results[core]["outT"])  # [KB, 128, t_steps, BL]
        outs.append(
            np.ascontiguousarray(ot.transpose(3, 2, 0, 1)).reshape(BL, t_steps, H)
        )
    return np.concatenate(outs, axis=0), res


def kernel(x, w_i, b_i, w_h, b_h):
    out, _ = run(
        np.asarray(x, dtype=np.float32),
        np.asarray(w_i, dtype=np.float32),
        np.asarray(b_i, dtype=np.float32),
        np.asarray(w_h, dtype=np.float32),
        np.asarray(b_h, dtype=np.float32),
    )
    return out


# revision 10
# speedup vs baseline: 1.0678x; 1.0678x over previous
"""GRU kernel for Trainium2 (8 NeuronCores, data-parallel over batch).

Problem shapes (hardcoded): x [B=256, T=512, D=256], w_i [256, 1536],
w_h [512, 1536], b_i/b_h [1536]; out [256, 512, 512] fp32.

Strategy:
  - Shard batch B across 8 cores (32 samples each), replicate weights.
  - On-chip layout is "transposed/folded": the 3H gate axis lives on the
    128 partitions (12 m-tiles of 128), batch on the free axis, so the
    per-step gate arithmetic uses all 128 vector lanes.
  - Per recurrence step: hg = h @ w_h via 48 bf16 matmuls with w_h tiles
    as the (self-loading) stationary operand and h^T [128, 32] as the
    moving operand, accumulating into PSUM.  The x-projection for the
    z/r gates is added into the same PSUM with a single identity matmul,
    so the sigmoids read PSUM directly.
  - The matmul burst runs kb-outer so the first waves only need the
    first half of the freshly cast h; the gate chain is split into
    halves to pipeline Vector against Scalar.
  - xg = x @ w_i (+ biases) for the NEXT 32-step chunk is computed by
    N=512 matmuls placed right after each burst, hiding in the
    dependency-wait gap of the sequential recurrence.
  - Host pre-transposes x / folds weights to bf16 and un-transposes the
    output (cheap numpy, not on the device critical path).
"""

import sys

if "/opt/trn_rl_repo" not in sys.path:
    sys.path.insert(0, "/opt/trn_rl_repo")

from contextlib import ExitStack

import ml_dtypes
import numpy as np

import concourse.bass as bass
import concourse.tile as tile
from concourse import bacc, mybir
from concourse.bass_utils import run_bass_kernel_spmd

B, T, D, H = 256, 512, 256, 512
NCORES = 8
BL = B // NCORES  # 32 samples per core
G3 = 3 * H  # 1536
MT = G3 // 128  # 12 m-tiles over the gate axis
KB = H // 128  # 4 fold blocks of h (contraction for w_h)
KD = D // 128  # 2 fold blocks of d (contraction for w_i)

F32 = mybir.dt.float32
BF16 = mybir.dt.bfloat16
AF = mybir.ActivationFunctionType
ALU = mybir.AluOpType
BF16NP = ml_dtypes.bfloat16

_cache = {}

import os as _os

VARIANT = _os.environ.get("GRU_VARIANT", "noid")
ORDER = _os.environ.get("GRU_ORDER", "mt")      # kb | mt
EVAC = _os.environ.get("GRU_EVAC", "mix")        # mix | s
COPYE = _os.environ.get("GRU_COPY", "v")         # v | s
PSXG = int(_os.environ.get("GRU_PSXG", "4"))



def _build_program(t_steps: int, tc: int, with_bhn: bool):
    """Emit + compile the SPMD Bass program for one core's shard."""
    assert t_steps % tc == 0
    nchunk = t_steps // tc
    ncols_chunk = tc * BL
    XN = min(512, ncols_chunk)  # moving N per xg matmul
    ncol = ncols_chunk // XN  # N-slices per (chunk, m-tile)
    tper = XN // BL  # timesteps covered by one N-slice

    nc = bacc.Bacc(
        "TRN2",
        target_bir_lowering=False,
        debug=False,
        enable_asserts=False,
        num_devices=NCORES,
    )

    xt_d = nc.dram_tensor("xt", [KD, 128, t_steps * BL], BF16, kind="ExternalInput").ap()
    whp_d = nc.dram_tensor("whp", [KB, 128, G3], BF16, kind="ExternalInput").ap()
    wip_d = nc.dram_tensor("wip", [KD, 128, G3], BF16, kind="ExternalInput").ap()
    xb_d = nc.dram_tensor("xb", [128, MT], F32, kind="ExternalInput").ap()
    bhn_d = nc.dram_tensor("bhn", [128, KB], F32, kind="ExternalInput").ap()
    id_d = nc.dram_tensor("ident", [128, 128], BF16, kind="ExternalInput").ap()
    out_d = nc.dram_tensor("outT", [KB, 128, t_steps, BL], F32, kind="ExternalOutput").ap()

    with tile.TileContext(nc) as tc_ctx, ExitStack() as ctx:
        consts = ctx.enter_context(tc_ctx.tile_pool(name="consts", bufs=1))
        xtp = ctx.enter_context(tc_ctx.tile_pool(name="xt", bufs=3))
        xgp = ctx.enter_context(tc_ctx.tile_pool(name="xg", bufs=2))
        outp = ctx.enter_context(tc_ctx.tile_pool(name="outc", bufs=2))
        hbfp = ctx.enter_context(tc_ctx.tile_pool(name="hbf", bufs=3))
        tmps = ctx.enter_context(tc_ctx.tile_pool(name="tmps", bufs=3))
        ps_zr = ctx.enter_context(tc_ctx.tile_pool(name="ps_zr", bufs=2, space="PSUM"))
        ps_n = ctx.enter_context(tc_ctx.tile_pool(name="ps_n", bufs=2, space="PSUM"))
        ps_xg = ctx.enter_context(tc_ctx.tile_pool(name="ps_xg", bufs=PSXG, space="PSUM"))

        # --- resident constants -------------------------------------------
        wh_sb = consts.tile([128, KB, G3], BF16)
        wi_sb = consts.tile([128, KD, G3], BF16)
        xb_sb = consts.tile([128, MT], F32)
        bhn_sb = consts.tile([128, KB], F32)
        id_sb = consts.tile([128, 128], BF16)
        zero_f32 = consts.tile([128, KB, BL], F32)
        zero_bf = consts.tile([128, KB, BL], BF16)

        for kb in range(KB):
            nc.sync.dma_start(wh_sb[:, kb, :], whp_d[kb])
        for kd in range(KD):
            nc.sync.dma_start(wi_sb[:, kd, :], wip_d[kd])
        nc.sync.dma_start(xb_sb[:], xb_d[:])
        nc.sync.dma_start(bhn_sb[:], bhn_d[:])
        nc.sync.dma_start(id_sb[:], id_d[:])
        nc.vector.memset(zero_f32[:], 0.0)
        nc.vector.memset(zero_bf[:], 0.0)

        # ---- xg GEMM emitter (one (mt, j) N-slice of a chunk) ------------
        evac_flip = [0]

        def emit_xg_group(xg_tile, xt_tile, mt, j):
            pt = ps_xg.tile([128, XN], F32, tag="psxg")
            for kd in range(KD):
                nc.tensor.matmul(
                    pt[:],
                    lhsT=wi_sb[:, kd, mt * 128 : (mt + 1) * 128],
                    rhs=xt_tile[:, kd, j * XN : (j + 1) * XN],
                    start=(kd == 0),
                    stop=(kd == KD - 1),
                )
            dst = xg_tile[:, mt, j * tper : (j + 1) * tper, :]
            # evacuate + bias (b_i, plus b_h for the z/r tiles) -> bf16,
            # alternating between Scalar and Vector to balance load
            if EVAC == "s" or evac_flip[0] % 2 == 0:
                nc.scalar.activation(dst, pt[:], AF.Identity, bias=xb_sb[:, mt : mt + 1])
            else:
                nc.vector.tensor_scalar_add(dst, pt[:], xb_sb[:, mt : mt + 1])
            evac_flip[0] += 1

        def load_xt(c):
            t0 = c * tc
            xt_tile = xtp.tile([128, KD, ncols_chunk], BF16, tag="xt")
            for kd in range(KD):
                nc.sync.dma_start(
                    xt_tile[:, kd, :], xt_d[kd, :, t0 * BL : (t0 + tc) * BL]
                )
            return xt_tile

        # preamble: xt for chunks 0/1, xg for chunk 0
        xt_cur = load_xt(0)
        xt_next = load_xt(1) if nchunk > 1 else None
        xg_cur = xgp.tile([128, MT, tc, BL], BF16, tag="xg")
        for mt in range(MT):
            for j in range(ncol):
                emit_xg_group(xg_cur, xt_cur, mt, j)

        h_bf = zero_bf
        h_f32 = zero_f32

        # kb3 wave order: r tiles first (frees SIG_r early), then z, then n
        KB3_ORDER = [4, 5, 6, 7, 0, 1, 2, 3, 8, 9, 10, 11]

        for c in range(nchunk):
            outc = outp.tile([128, KB, tc, BL], F32, tag="outc")
            groups = [(mt, j) for mt in range(MT) for j in range(ncol)]
            if c + 1 < nchunk:
                xg_next = xgp.tile([128, MT, tc, BL], BF16, tag="xg")
                xt_after = load_xt(c + 2) if c + 2 < nchunk else None
            else:
                xg_next = None

            for tt in range(tc):
                # ---- hg PSUM: xg_zr via identity matmul, then h @ w_h ----
                _variant = VARIANT
                pzr = ps_zr.tile([128, 8, BL], F32, tag="pzr")
                pn = ps_n.tile([128, KB, BL], F32, tag="pn")
                if _variant == "idmm":
                    for mt in range(8):
                        nc.tensor.matmul(
                            pzr[:, mt, :],
                            lhsT=id_sb[:],
                            rhs=xg_cur[:, mt, tt, :],
                            start=True,
                            stop=False,
                            skip_group_check=True,
                        )
                if ORDER == "kb":
                    burst = [(kb, mt) for kb in range(KB)
                             for mt in (KB3_ORDER if kb == KB - 1 else range(MT))]
                else:
                    burst = [(kb, mt) for mt in range(MT) for kb in range(KB)]
                for kb, mt in burst:
                    dst = pzr[:, mt, :] if mt < 8 else pn[:, mt - 8, :]
                    nc.tensor.matmul(
                        dst,
                        lhsT=wh_sb[:, kb, mt * 128 : (mt + 1) * 128],
                        rhs=h_bf[:, kb, :],
                        start=(kb == 0 and (mt >= 8 or _variant != "idmm")),
                        stop=(kb == KB - 1),
                        skip_group_check=True,
                    )
                # xg work for the next chunk rides in the dependency gap
                if xg_next is not None:
                    lo = len(groups) * tt // tc
                    hi = len(groups) * (tt + 1) // tc
                    for mt, j in groups[lo:hi]:
                        emit_xg_group(xg_next, xt_next, mt, j)

                # ---- gates (halves pipelined: blk 0:2 then 2:4) ----------
                zr = tmps.tile([128, 8, BL], F32, tag="zr")
                zc = tmps.tile([128, KB, BL], F32, tag="zc")
                tn = tmps.tile([128, KB, BL], F32, tag="tn")
                npre = tmps.tile([128, KB, BL], F32, tag="npre")
                nv = tmps.tile([128, KB, BL], F32, tag="nv")
                zh = tmps.tile([128, KB, BL], F32, tag="zh")
                w1 = tmps.tile([128, KB, BL], F32, tag="w1")
                hn = outc[:, :, tt, :]
                h_new_bf = hbfp.tile([128, KB, BL], BF16, tag="hbf")

                # Scalar queue: r first, then 1-z, z, then the tanh halves
                if _variant == "idmm":
                    zsrc = pzr
                else:
                    zsrc = tmps.tile([128, 8, BL], F32, tag="zrpre")
                    nc.vector.tensor_add(zsrc[:], pzr[:], xg_cur[:, 0:8, tt, :])
                nc.scalar.activation(zr[:, 4:8, :], zsrc[:, 4:8, :], AF.Sigmoid)
                nc.scalar.activation(zc[:], zsrc[:, 0:4, :], AF.Sigmoid, scale=-1.0)
                nc.scalar.activation(zr[:, 0:4, :], zsrc[:, 0:4, :], AF.Sigmoid)

                for h0, h1 in ((0, 2), (2, 4)):
                    if with_bhn:
                        for kb in range(h0, h1):
                            nc.vector.scalar_tensor_tensor(
                                tn[:, kb, :], pn[:, kb, :],
                                bhn_sb[:, kb : kb + 1], zr[:, 4 + kb, :],
                                op0=ALU.add, op1=ALU.mult,
                            )
                    else:
                        nc.vector.tensor_mul(
                            tn[:, h0:h1, :], pn[:, h0:h1, :], zr[:, 4 + h0 : 4 + h1, :]
                        )
                    nc.vector.tensor_add(
                        npre[:, h0:h1, :], tn[:, h0:h1, :], xg_cur[:, 8 + h0 : 8 + h1, tt, :]
                    )
                    nc.scalar.activation(
                        nv[:, h0:h1, :], npre[:, h0:h1, :], AF.Tanh
                    )

                for h0, h1 in ((0, 2), (2, 4)):
                    nc.vector.tensor_mul(zh[:, h0:h1, :], zr[:, h0:h1, :], h_f32[:, h0:h1, :])
                    nc.vector.tensor_mul(w1[:, h0:h1, :], zc[:, h0:h1, :], nv[:, h0:h1, :])
                    nc.vector.tensor_add(hn[:, h0:h1, :], w1[:, h0:h1, :], zh[:, h0:h1, :])
                    if COPYE == "v":
                        nc.vector.tensor_copy(h_new_bf[:, h0:h1, :], hn[:, h0:h1, :])
                    else:
                        nc.scalar.activation(h_new_bf[:, h0:h1, :], hn[:, h0:h1, :], AF.Copy)

                h_f32 = hn
                h_bf = h_new_bf

            # ---- store chunk, rotate buffers ----
            t0 = c * tc
            for kb in range(KB):
                nc.sync.dma_start(out_d[kb, :, t0 : t0 + tc, :], outc[:, kb, :, :])
            xg_cur = xg_next
            xt_cur = xt_next
            if c + 1 < nchunk:
                xt_next = xt_after

    nc.compile()
    return nc


def _get_program(t_steps: int, tc: int, with_bhn: bool):
    key = (t_steps, tc, with_bhn)
    if key not in _cache:
        _cache[key] = _build_program(t_steps, tc, with_bhn)
    return _cache[key]


def _prep_core_inputs(x, w_i, b_i, w_h, b_h, core, t_steps):
    xc = x[core * BL : (core + 1) * BL, :t_steps, :]  # [BL, T, D]
    xt = np.ascontiguousarray(xc.transpose(2, 1, 0)).reshape(KD, 128, t_steps * BL)
    xb = b_i.astype(np.float64).copy()
    xb[: 2 * H] += b_h[: 2 * H].astype(np.float64)
    return {
        "xt": xt.astype(BF16NP),
        "whp": w_h.reshape(KB, 128, G3).astype(BF16NP),
        "wip": w_i.reshape(KD, 128, G3).astype(BF16NP),
        "xb": np.ascontiguousarray(xb.astype(np.float32).reshape(MT, 128).T),
        "bhn": np.ascontiguousarray(b_h[2 * H :].reshape(KB, 128).T.astype(np.float32)),
        "ident": np.eye(128, dtype=np.float32).astype(BF16NP),
    }


def run(x, w_i, b_i, w_h, b_h, t_steps=T, tc=32, trace=False):
    with_bhn = bool(np.any(b_h[2 * H :] != 0))
    nc = _get_program(t_steps, tc, with_bhn)
    in_maps = [
        _prep_core_inputs(x, w_i, b_i, w_h, b_h, core, t_steps)
        for core in range(NCORES)
    ]
    res = run_bass_kernel_spmd(
        nc, in_maps, core_ids=list(range(NCORES)), trace=trace
    )
    outs = []
    for core in range(NCORES):
        ot = np.asarray(res.results[core]["outT"])  # [KB, 128, t_steps, BL]
        outs.append(
            np.ascontiguousarray(ot.transpose(3, 2, 0, 1)).reshape(BL, t_steps, H)
        )
    return np.concatenate(outs, axis=0), res


def kernel(x, w_i, b_i, w_h, b_h):
    out, _ = run(
        np.asarray(x, dtype=np.float32),
        np.asarray(w_i, dtype=np.float32),
        np.asarray(b_i, dtype=np.float32),
        np.asarray(w_h, dtype=np.float32),
        np.asarray(b_h, dtype=np.float32),
    )
    return out


# revision 11
# speedup vs baseline: 1.0942x; 1.0248x over previous
"""GRU kernel for Trainium2 (8 NeuronCores, data-parallel over batch).

Problem shapes (hardcoded): x [B=256, T=512, D=256], w_i [256, 1536],
w_h [512, 1536], b_i/b_h [1536]; out [256, 512, 512] fp32.

Strategy:
  - Shard batch B across 8 cores (32 samples each), replicate weights.
  - On-chip layout is "transposed/folded": the 3H gate axis lives on the
    128 partitions (12 m-tiles of 128), batch on the free axis, so the
    per-step gate arithmetic uses all 128 vector lanes.
  - Per recurrence step: hg = h @ w_h via 48 bf16 matmuls with w_h tiles
    as the (self-loading) stationary operand and h^T [128, 32] as the
    moving operand, accumulating into PSUM.  The x-projection for the
    z/r gates is added into the same PSUM with a single identity matmul,
    so the sigmoids read PSUM directly.
  - The matmul burst runs kb-outer so the first waves only need the
    first half of the freshly cast h; the gate chain is split into
    halves to pipeline Vector against Scalar.
  - xg = x @ w_i (+ biases) for the NEXT 32-step chunk is computed by
    N=512 matmuls placed right after each burst, hiding in the
    dependency-wait gap of the sequential recurrence.
  - Host pre-transposes x / folds weights to bf16 and un-transposes the
    output (cheap numpy, not on the device critical path).
"""

import sys

if "/opt/trn_rl_repo" not in sys.path:
    sys.path.insert(0, "/opt/trn_rl_repo")

from contextlib import ExitStack

import ml_dtypes
import numpy as np

import concourse.bass as bass
import concourse.tile as tile
from concourse import bacc, mybir
from concourse.bass_utils import run_bass_kernel_spmd

B, T, D, H = 256, 512, 256, 512
NCORES = 8
BL = B // NCORES  # 32 samples per core
G3 = 3 * H  # 1536
MT = G3 // 128  # 12 m-tiles over the gate axis
KB = H // 128  # 4 fold blocks of h (contraction for w_h)
KD = D // 128  # 2 fold blocks of d (contraction for w_i)

F32 = mybir.dt.float32
BF16 = mybir.dt.bfloat16
AF = mybir.ActivationFunctionType
ALU = mybir.AluOpType
BF16NP = ml_dtypes.bfloat16

_cache = {}

import os as _os

VARIANT = _os.environ.get("GRU_VARIANT", "noid")
ORDER = _os.environ.get("GRU_ORDER", "mt")      # kb | mt
EVAC = _os.environ.get("GRU_EVAC", "mix")        # mix | s
COPYE = _os.environ.get("GRU_COPY", "v")         # v | s
PSXG = int(_os.environ.get("GRU_PSXG", "4"))



def _build_program(t_steps: int, tc: int, with_bhn: bool):
    """Emit + compile the SPMD Bass program for one core's shard."""
    assert t_steps % tc == 0
    nchunk = t_steps // tc
    ncols_chunk = tc * BL
    XN = min(256, ncols_chunk)  # moving N per xg matmul
    ncol = ncols_chunk // XN  # N-slices per (chunk, m-tile)
    tper = XN // BL  # timesteps covered by one N-slice

    nc = bacc.Bacc(
        "TRN2",
        target_bir_lowering=False,
        debug=False,
        enable_asserts=False,
        num_devices=NCORES,
    )

    xt_d = nc.dram_tensor("xt", [KD, 128, t_steps * BL], BF16, kind="ExternalInput").ap()
    whp_d = nc.dram_tensor("whp", [KB, 128, G3], BF16, kind="ExternalInput").ap()
    wip_d = nc.dram_tensor("wip", [KD, 128, G3], BF16, kind="ExternalInput").ap()
    xb_d = nc.dram_tensor("xb", [128, MT], F32, kind="ExternalInput").ap()
    bhn_d = nc.dram_tensor("bhn", [128, KB], F32, kind="ExternalInput").ap()
    id_d = nc.dram_tensor("ident", [128, 128], BF16, kind="ExternalInput").ap()
    out_d = nc.dram_tensor("outT", [KB, 128, t_steps, BL], F32, kind="ExternalOutput").ap()

    with tile.TileContext(nc) as tc_ctx, ExitStack() as ctx:
        consts = ctx.enter_context(tc_ctx.tile_pool(name="consts", bufs=1))
        xtp = ctx.enter_context(tc_ctx.tile_pool(name="xt", bufs=3))
        xgp = ctx.enter_context(tc_ctx.tile_pool(name="xg", bufs=2))
        outp = ctx.enter_context(tc_ctx.tile_pool(name="outc", bufs=2))
        hbfp = ctx.enter_context(tc_ctx.tile_pool(name="hbf", bufs=4))
        tmps = ctx.enter_context(tc_ctx.tile_pool(name="tmps", bufs=4))
        ps_zr = ctx.enter_context(tc_ctx.tile_pool(name="ps_zr", bufs=2, space="PSUM"))
        ps_n = ctx.enter_context(tc_ctx.tile_pool(name="ps_n", bufs=2, space="PSUM"))
        ps_xg = ctx.enter_context(tc_ctx.tile_pool(name="ps_xg", bufs=PSXG, space="PSUM"))

        # --- resident constants -------------------------------------------
        wh_sb = consts.tile([128, KB, G3], BF16)
        wi_sb = consts.tile([128, KD, G3], BF16)
        xb_sb = consts.tile([128, MT], F32)
        bhn_sb = consts.tile([128, KB], F32)
        id_sb = consts.tile([128, 128], BF16)
        zero_f32 = consts.tile([128, KB, BL], F32)
        zero_bf = consts.tile([128, KB, BL], BF16)

        for kb in range(KB):
            nc.sync.dma_start(wh_sb[:, kb, :], whp_d[kb])
        for kd in range(KD):
            nc.sync.dma_start(wi_sb[:, kd, :], wip_d[kd])
        nc.sync.dma_start(xb_sb[:], xb_d[:])
        nc.sync.dma_start(bhn_sb[:], bhn_d[:])
        nc.sync.dma_start(id_sb[:], id_d[:])
        nc.vector.memset(zero_f32[:], 0.0)
        nc.vector.memset(zero_bf[:], 0.0)

        # ---- xg GEMM emitter (one (mt, j) N-slice of a chunk) ------------
        evac_flip = [0]

        def emit_xg_group(xg_tile, xt_tile, mt, j):
            pt = ps_xg.tile([128, XN], F32, tag="psxg")
            for kd in range(KD):
                nc.tensor.matmul(
                    pt[:],
                    lhsT=wi_sb[:, kd, mt * 128 : (mt + 1) * 128],
                    rhs=xt_tile[:, kd, j * XN : (j + 1) * XN],
                    start=(kd == 0),
                    stop=(kd == KD - 1),
                )
            dst = xg_tile[:, mt, j * tper : (j + 1) * tper, :]
            # evacuate + bias (b_i, plus b_h for the z/r tiles) -> bf16,
            # alternating between Scalar and Vector to balance load
            if EVAC == "s" or evac_flip[0] % 2 == 0:
                nc.scalar.activation(dst, pt[:], AF.Identity, bias=xb_sb[:, mt : mt + 1])
            else:
                nc.vector.tensor_scalar_add(dst, pt[:], xb_sb[:, mt : mt + 1])
            evac_flip[0] += 1

        def load_xt(c):
            t0 = c * tc
            xt_tile = xtp.tile([128, KD, ncols_chunk], BF16, tag="xt")
            for kd in range(KD):
                nc.sync.dma_start(
                    xt_tile[:, kd, :], xt_d[kd, :, t0 * BL : (t0 + tc) * BL]
                )
            return xt_tile

        # preamble: xt for chunks 0/1, xg for chunk 0
        xt_cur = load_xt(0)
        xt_next = load_xt(1) if nchunk > 1 else None
        xg_cur = xgp.tile([128, MT, tc, BL], BF16, tag="xg")
        for mt in range(MT):
            for j in range(ncol):
                emit_xg_group(xg_cur, xt_cur, mt, j)

        h_bf = zero_bf
        h_f32 = zero_f32

        # kb3 wave order: r tiles first (frees SIG_r early), then z, then n
        KB3_ORDER = [4, 5, 6, 7, 0, 1, 2, 3, 8, 9, 10, 11]

        for c in range(nchunk):
            outc = outp.tile([128, KB, tc, BL], F32, tag="outc")
            groups = [(mt, j) for mt in range(MT) for j in range(ncol)]
            if c + 1 < nchunk:
                xg_next = xgp.tile([128, MT, tc, BL], BF16, tag="xg")
                xt_after = load_xt(c + 2) if c + 2 < nchunk else None
            else:
                xg_next = None

            for tt in range(tc):
                # ---- hg PSUM: xg_zr via identity matmul, then h @ w_h ----
                _variant = VARIANT
                pzr = ps_zr.tile([128, 8, BL], F32, tag="pzr")
                pn = ps_n.tile([128, KB, BL], F32, tag="pn")
                if _variant == "idmm":
                    for mt in range(8):
                        nc.tensor.matmul(
                            pzr[:, mt, :],
                            lhsT=id_sb[:],
                            rhs=xg_cur[:, mt, tt, :],
                            start=True,
                            stop=False,
                            skip_group_check=True,
                        )
                if ORDER == "kb":
                    burst = [(kb, mt) for kb in range(KB)
                             for mt in (KB3_ORDER if kb == KB - 1 else range(MT))]
                else:
                    burst = [(kb, mt) for mt in range(MT) for kb in range(KB)]
                for kb, mt in burst:
                    dst = pzr[:, mt, :] if mt < 8 else pn[:, mt - 8, :]
                    nc.tensor.matmul(
                        dst,
                        lhsT=wh_sb[:, kb, mt * 128 : (mt + 1) * 128],
                        rhs=h_bf[:, kb, :],
                        start=(kb == 0 and (mt >= 8 or _variant != "idmm")),
                        stop=(kb == KB - 1),
                        skip_group_check=True,
                    )
                # xg work for the next chunk rides in the dependency gap
                if xg_next is not None:
                    lo = len(groups) * tt // tc
                    hi = len(groups) * (tt + 1) // tc
                    for mt, j in groups[lo:hi]:
                        emit_xg_group(xg_next, xt_next, mt, j)

                # ---- gates (halves pipelined: blk 0:2 then 2:4) ----------
                zr = tmps.tile([128, 8, BL], F32, tag="zr")
                zc = tmps.tile([128, KB, BL], F32, tag="zc")
                tn = tmps.tile([128, KB, BL], F32, tag="tn")
                npre = tmps.tile([128, KB, BL], F32, tag="npre")
                nv = tmps.tile([128, KB, BL], F32, tag="nv")
                zh = tmps.tile([128, KB, BL], F32, tag="zh")
                w1 = tmps.tile([128, KB, BL], F32, tag="w1")
                hn = outc[:, :, tt, :]
                h_new_bf = hbfp.tile([128, KB, BL], BF16, tag="hbf")

                # Scalar queue: r first, then 1-z, z, then the tanh halves
                if _variant == "idmm":
                    zsrc = pzr
                else:
                    zsrc = tmps.tile([128, 8, BL], F32, tag="zrpre")
                    nc.vector.tensor_add(
                        zsrc[:, 4:8, :], pzr[:, 4:8, :], xg_cur[:, 4:8, tt, :]
                    )
                    nc.vector.tensor_add(
                        zsrc[:, 0:4, :], pzr[:, 0:4, :], xg_cur[:, 0:4, tt, :]
                    )
                nc.scalar.activation(zr[:, 4:8, :], zsrc[:, 4:8, :], AF.Sigmoid)
                nc.scalar.activation(zc[:], zsrc[:, 0:4, :], AF.Sigmoid, scale=-1.0)
                nc.scalar.activation(zr[:, 0:4, :], zsrc[:, 0:4, :], AF.Sigmoid)

                for h0, h1 in ((0, 2), (2, 4)):
                    if with_bhn:
                        for kb in range(h0, h1):
                            nc.vector.scalar_tensor_tensor(
                                tn[:, kb, :], pn[:, kb, :],
                                bhn_sb[:, kb : kb + 1], zr[:, 4 + kb, :],
                                op0=ALU.add, op1=ALU.mult,
                            )
                    else:
                        nc.vector.tensor_mul(
                            tn[:, h0:h1, :], pn[:, h0:h1, :], zr[:, 4 + h0 : 4 + h1, :]
                        )
                    nc.vector.tensor_add(
                        npre[:, h0:h1, :], tn[:, h0:h1, :], xg_cur[:, 8 + h0 : 8 + h1, tt, :]
                    )
                    nc.scalar.activation(
                        nv[:, h0:h1, :], npre[:, h0:h1, :], AF.Tanh
                    )

                for h0, h1 in ((0, 2), (2, 4)):
                    nc.vector.tensor_mul(zh[:, h0:h1, :], zr[:, h0:h1, :], h_f32[:, h0:h1, :])
                    nc.vector.tensor_mul(w1[:, h0:h1, :], zc[:, h0:h1, :], nv[:, h0:h1, :])
                    nc.vector.tensor_add(hn[:, h0:h1, :], w1[:, h0:h1, :], zh[:, h0:h1, :])
                    if COPYE == "v":
                        nc.vector.tensor_copy(h_new_bf[:, h0:h1, :], hn[:, h0:h1, :])
                    else:
                        nc.scalar.activation(h_new_bf[:, h0:h1, :], hn[:, h0:h1, :], AF.Copy)

                h_f32 = hn
                h_bf = h_new_bf

            # ---- store chunk, rotate buffers ----
            t0 = c * tc
            for kb in range(KB):
                nc.sync.dma_start(out_d[kb, :, t0 : t0 + tc, :], outc[:, kb, :, :])
            xg_cur = xg_next
            xt_cur = xt_next
            if c + 1 < nchunk:
                xt_next = xt_after

    nc.compile()
    return nc


def _get_program(t_steps: int, tc: int, with_bhn: bool):
    key = (t_steps, tc, with_bhn)
    if key not in _cache:
        _cache[key] = _build_program(t_steps, tc, with_bhn)
    return _cache[key]


def _prep_core_inputs(x, w_i, b_i, w_h, b_h, core, t_steps):
    xc = x[core * BL : (core + 1) * BL, :t_steps, :]  # [BL, T, D]
    xt = np.ascontiguousarray(xc.transpose(2, 1, 0)).reshape(KD, 128, t_steps * BL)
    xb = b_i.astype(np.float64).copy()
    xb[: 2 * H] += b_h[: 2 * H].astype(np.float64)
    return {
        "xt": xt.astype(BF16NP),
        "whp": w_h.reshape(KB, 128, G3).astype(BF16NP),
        "wip": w_i.reshape(KD, 128, G3).astype(BF16NP),
        "xb": np.ascontiguousarray(xb.astype(np.float32).reshape(MT, 128).T),
        "bhn": np.ascontiguousarray(b_h[2 * H :].reshape(KB, 128).T.astype(np.float32)),
        "ident": np.eye(128, dtype=np.float32).astype(BF16NP),
    }


def run(x, w_i, b_i, w_h, b_h, t_steps=T, tc=32, trace=False):
    with_bhn = bool(np.any(b_h[2 * H :] != 0))
    nc = _get_program(t_steps, tc, with_bhn)
    in_maps = [
        _prep_core_inputs(x, w_i, b_i, w_h, b_h, core, t_steps)
        for core in range(NCORES)
    ]
    res = run_bass_kernel_spmd(
        nc, in_maps, core_ids=list(range(NCORES)), trace=trace
    )
    outs = []
    for core in range(NCORES):
        ot = np.asarray(res.results[core]["outT"])  # [KB, 128, t_steps, BL]
        outs.append(
            np.ascontiguousarray(ot.transpose(3, 2, 0, 1)).reshape(BL, t_steps, H)
        )
    return np.concatenate(outs, axis=0), res


def kernel(x, w_i, b_i, w_h, b_h):
    out, _ = run(
        np.asarray(x, dtype=np.float32),
        np.asarray(w_i, dtype=np.float32),
        np.asarray(b_i, dtype=np.float32),
        np.asarray(w_h, dtype=np.float32),
        np.asarray(b_h, dtype=np.float32),
    )
    return out
